# revision 5
# baseline (speedup 1.0000x reference)
"""Grouped per-sample MLP (conv1d groups=B) + GroupSwish + softmax, on 8 NeuronCores.

Data-parallel over the group/batch axis B=256: 32 groups per core,
processed as 8 quads of 4 groups packed into the 128-partition dim.

Per group g: h = W1[g] @ x[g] + b1[g]; GroupSwish; o = W2[g] @ h + b2[g];
softmax over the flattened [C*L] logits.

The kernel is HBM-stream-bound (~290 GB/s/core under 8-core load, 14.1MB
per core => ~48us stream floor); every design choice keeps the stream
saturated and the post-stream tail short:
  - x and W1 ship as fp8e4m3, W2/swish as fp16, out as bf16 (upcast on
    host). End-to-end rel err ~9e-3 vs the 2e-2 gate.
  - The two DMA queues (sync HWDGE + gpsimd SWDGE) carry byte-identical
    halves of every quad's x (chunks 0-2 + half of 3 | other half of 3 +
    chunks 4-6, each marshalled contiguous as 7168B-per-partition rows),
    plus half of W1 each, so both queues drain in lockstep and the last
    quad's data is not skewed to one queue.
  - Each half is further split into granules (2 per half mid-stream,
    finer on the last quad) so W1 matmuls consume x as it lands; after
    the final byte only the last granule's matmuls remain.
  - Emission order per iteration interleaves the previous quads' late
    stages between matmul granule groups, so in-order engine queues
    never park a ready instruction behind a stalled matmul:
    PE:  [mmA0(q)] [W2(q-1)] [mmB0(q)] [tot(q-2)] [mmA1(q)] [mmB1(q)]
    DVE: [recip(q-2)] [mul(q-3)] [u(q)] [sw(q)]
    ACT: [exp(q-1)] [tanh(q)]
  - One store DMA per quad ([4,10,512] partition-strided pattern),
    alternating engines.
  - All of W1 stays resident in SBUF; softplus(beta), b1 folding and
    W2/1.1 folding are done host-side. W2 is padded to [Z, 32] with
    zeros so all 128 partitions of the logits PSUM are written (pad rows
    get exp(-30) ~ 0); softmax normalization via one [128,128]
    block-mask matmul yielding per-partition group totals.
"""

import os
import ml_dtypes
import numpy as np
from contextlib import ExitStack

import concourse.mybir as mybir
import concourse.tile as tile
from concourse import bacc
from concourse.bass_utils import run_bass_kernel_spmd

B, X, Z, C, L = 256, 784, 32, 10, 512
NCORE = 8
GPC = B // NCORE  # 32 groups per core
NQ = GPC // 4  # 8 quads per core
KC = 112  # K-chunk size (7 * 112 = 784)
NCH = 7
P = 128
HB = 7168  # bytes per partition per x half (3.5 chunks * 4 groups * 512)
F32 = mybir.dt.float32
F16 = mybir.dt.float16
F8 = mybir.dt.float8e4
BF16 = mybir.dt.bfloat16

# (chunk c, group j) -> (half, byte offset) in the half's free dim.
# half A: c0,c1,c2 all j; c3 j0,j1.  half B: c3 j2,j3; c4,c5,c6 all j.
def _cj_off(c, j):
    if c < 3:
        return 0, c * 2048 + j * 512
    if c == 3:
        return (0, 6144 + j * 512) if j < 2 else (1, (j - 2) * 512)
    return 1, 1024 + (c - 4) * 2048 + j * 512


DEFAULT_CFG = dict(
    x_bufs=4,
    s_bufs=4,
    h_bufs=3,
    o_bufs=2,
    x_engines=("sync", "gpsimd"),
    out_engines=("gpsimd", "sync"),
    const_engine="gpsimd",
    mid_splits=2,   # granules per half for mid-stream quads
    last_splits=4,  # granules per half for the last quad
)

_CACHE: dict = {}


def _eng(nc, name):
    return getattr(nc, name)


def _build(cfg=DEFAULT_CFG):
    nc = bacc.Bacc("TRN2", target_bir_lowering=False, debug=False)

    xa = nc.dram_tensor("xa", [NQ, KC, HB], F8, kind="ExternalInput").ap()
    xb = nc.dram_tensor("xb", [NQ, KC, HB], F8, kind="ExternalInput").ap()
    w1q = nc.dram_tensor(
        "w1q", [KC, NQ * 4 * NCH * Z], F8, kind="ExternalInput"
    ).ap()
    # w2q[32j+z, 32q+m] = W2[4q+j, m, z]/1.1 (m<C), 0 for m>=C
    w2q = nc.dram_tensor("w2q", [P, NQ * 32], F16, kind="ExternalInput").ap()
    # scal[:, 0:NQ]=b1, [NQ:2NQ]=softplus(beta)/2, [2NQ:3NQ]=sp*b1/2, [3NQ:4NQ]=b2
    scalq = nc.dram_tensor("scalq", [P, 4 * NQ], F32, kind="ExternalInput").ap()
    # maskb[p, m] = 1 iff p//32 == m//32 and p%32 < C
    maskb = nc.dram_tensor("maskb", [P, P], BF16, kind="ExternalInput").ap()
    out = nc.dram_tensor("out", [GPC, C, L], BF16, kind="ExternalOutput").ap()

    with tile.TileContext(nc) as tc, ExitStack() as ctx:
        consts = ctx.enter_context(tc.tile_pool(name="consts", bufs=1))
        xpool = ctx.enter_context(tc.tile_pool(name="x", bufs=2 * cfg["x_bufs"]))
        spool = ctx.enter_context(tc.tile_pool(name="act", bufs=cfg["s_bufs"]))
        hps = ctx.enter_context(
            tc.tile_pool(name="hps", bufs=cfg["h_bufs"], space="PSUM")
        )
        ops = ctx.enter_context(
            tc.tile_pool(name="ops", bufs=cfg["o_bufs"], space="PSUM")
        )
        tps = ctx.enter_context(tc.tile_pool(name="tps", bufs=2, space="PSUM"))

        ce = _eng(nc, cfg["const_engine"])
        xes = [_eng(nc, e) for e in cfg["x_engines"]]
        oes = [_eng(nc, e) for e in cfg["out_engines"]]

        # W1 resident in SBUF, half on each queue so the queues stay
        # byte-balanced.
        w1t = consts.tile([KC, NQ * 4 * NCH * Z], F8, name="w1t")
        half = NQ * 4 * NCH * Z // 2
        xes[0].dma_start(w1t[:, :half], w1q[:, :half])
        xes[1].dma_start(w1t[:, half:], w1q[:, half:])
        w2t = consts.tile([P, NQ * 32], F16, name="w2t")
        ce.dma_start(w2t[:], w2q)
        scalt = consts.tile([P, 4 * NQ], F32, name="scalt")
        ce.dma_start(scalt[:], scalq)
        maskt = consts.tile([P, P], BF16, name="maskt")
        ce.dma_start(maskt[:], maskb)
        b1t = scalt[:, 0:NQ]
        spht = scalt[:, NQ : 2 * NQ]
        spb1ht = scalt[:, 2 * NQ : 3 * NQ]
        b2t = scalt[:, 3 * NQ : 4 * NQ]

        hqs, swishes, expos, esums, invcs = {}, {}, {}, {}, {}

        def w1s(q, j, c):
            k = (q * 4 + j) * NCH + c
            return w1t[:, k * Z : (k + 1) * Z]

        def granules(q):
            """Granule byte boundaries (512-multiples) per half, and the
            (c, j) matmuls bucketed by which granule holds their x."""
            if q == NQ - 1:
                n = cfg["last_splits"]
            else:
                n = cfg["mid_splits"]
            # n equal-ish granules in units of 512B, summing to HB
            units = HB // 512  # 14
            per = [units // n + (1 if i < units % n else 0) for i in range(n)]
            bounds = []
            acc = 0
            for u in per:
                acc += u * 512
                bounds.append(acc)
            groups = [[[] for _ in range(n)], [[] for _ in range(n)]]
            for c in range(NCH):
                for j in range(4):
                    h, off = _cj_off(c, j)
                    gi = next(
                        i for i, b in enumerate(bounds) if off + 512 <= b
                    )
                    groups[h][gi].append((c, j))
            return n, bounds, groups

        def stage1_dma(q):
            """Issue the x granule DMAs for quad q; returns tiles+groups."""
            n, bounds, groups = granules(q)
            tiles = [[], []]
            src = [xa, xb]
            for h in range(2):
                lo = 0
                for gi in range(n):
                    hi = bounds[gi]
                    xt = xpool.tile(
                        [KC, hi - lo], F8, tag=f"x{h}_{gi}", name=f"x{q}_{h}_{gi}"
                    )
                    xes[h].dma_start(xt[:], src[h][q, :, lo:hi])
                    tiles[h].append((xt, lo))
                    lo = hi
            hq = hps.tile([P, L], F32, tag="h", name=f"h{q}")
            hqs[q] = hq
            return n, bounds, groups, tiles, hq

        def mm_block(q, h, gi, groups, tiles, hq, first, last):
            """W1 matmuls whose x lives (fully) in granule (h, gi)."""
            for c, j in groups[h][gi]:
                _, off = _cj_off(c, j)
                xt, lo = tiles[h][gi]
                off -= lo
                nc.tensor.matmul(
                    hq[32 * j : 32 * j + 32, :],
                    w1s(q, j, c),
                    xt[:, off : off + 512],
                    start=((h, gi, c) == first[j]),
                    stop=((h, gi, c) == last[j]),
                    tile_position=(0, 32 * j),
                    skip_group_check=True,
                )

        def stage_swish(q):
            """GroupSwish: ((h+b1)*0.5) * (1 + tanh(sp*(h+b1)/2))."""
            hq = hqs.pop(q)
            t = spool.tile([P, L], F32, tag="t", name=f"t{q}")
            nc.scalar.activation(
                t[:],
                hq[:],
                mybir.ActivationFunctionType.Tanh,
                bias=spb1ht[:, q : q + 1],
                scale=spht[:, q : q + 1],
            )
            u = spool.tile([P, L], F32, tag="u", name=f"u{q}")
            nc.vector.tensor_scalar(
                u[:],
                hq[:],
                b1t[:, q : q + 1],
                0.5,
                op0=mybir.AluOpType.add,
                op1=mybir.AluOpType.mult,
            )
            sw = spool.tile([P, L], F16, tag="sw", name=f"sw{q}")
            nc.vector.scalar_tensor_tensor(
                sw[:],
                t[:],
                1.0,
                u[:],
                op0=mybir.AluOpType.add,
                op1=mybir.AluOpType.mult,
            )
            swishes[q] = sw

        def stage2(q):
            """W2 matmuls + exp for quad q."""
            sw = swishes.pop(q)
            o = ops.tile([P, L], F32, tag="o", name=f"o{q}")
            for j in range(4):
                nc.tensor.matmul(
                    o[32 * j : 32 * j + 32, :],
                    w2t[32 * j : 32 * j + 32, q * 32 : (q + 1) * 32],
                    sw[32 * j : 32 * j + 32, :],
                    start=True,
                    stop=True,
                    tile_position=(32 * j, 32 * j),
                )
            expo = spool.tile([P, L], F32, tag="expo", name=f"e{q}")
            esum = spool.tile([P, 1], BF16, tag="esum", name=f"es{q}")
            with nc.allow_low_precision(reason="softmax denom, 2e-2 gate"):
                nc.scalar.activation(
                    expo[:],
                    o[:],
                    mybir.ActivationFunctionType.Exp,
                    bias=b2t[:, q : q + 1],
                    scale=1.0,
                    accum_out=esum[:],
                )
            expos[q] = expo
            esums[q] = esum

        def stage3a(q):
            """Per-group exp totals + reciprocal."""
            esum = esums.pop(q)
            tot = tps.tile([P, 1], F32, tag="tot", name=f"tot{q}")
            nc.tensor.matmul(tot[:], maskt[:], esum[:], start=True, stop=True)
            invc = spool.tile([P, 1], F32, tag="invc", name=f"ic{q}")
            nc.vector.reciprocal(invc[:], tot[:])
            invcs[q] = invc

        def stage3b(q):
            """Normalize + store: one [4,10,512] partition-strided DMA."""
            invc = invcs.pop(q)
            expo = expos.pop(q)
            res = spool.tile([P, L], BF16, tag="res", name=f"r{q}")
            nc.vector.tensor_scalar_mul(res[:], expo[:], invc[:])
            if os.environ.get("KERNEL_STORE4"):
                for j in range(4):
                    oes[(q + j) % 2].dma_start(
                        out[4 * q + j], res[32 * j : 32 * j + C, :]
                    )
            else:
                oes[q % 2].dma_start(
                    out[4 * q : 4 * q + 4],
                    res[:].rearrange("(j z) l -> j z l", j=4)[:, 0:C, :],
                )

        for q in range(NQ + 3):
            if q < NQ:
                n, bounds, groups, tiles, hq = stage1_dma(q)
                # emission order (likely-arrival): h0g0, h1g0, h0g1, ...
                order = [(h, gi) for gi in range(n) for h in range(2)]
                # start/stop flags follow emission order per col-group j
                first = {}
                last = {}
                for h, gi in order:
                    for c, j in groups[h][gi]:
                        key = (h, gi, c)
                        if j not in first:
                            first[j] = key
                        last[j] = key
                # interleave the previous quads' late stages between the
                # early granule blocks so in-order engine queues don't
                # park ready work behind stalled matmuls.
                inject = {
                    1: (lambda: stage2(q - 1)) if q >= 1 else None,
                    2: (lambda: stage3a(q - 2)) if q >= 2 else None,
                    3: (lambda: stage3b(q - 3)) if q >= 3 else None,
                }
                for bi, (h, gi) in enumerate(order):
                    mm_block(q, h, gi, groups, tiles, hq, first, last)
                    cb = inject.pop(bi + 1, None)
                    if cb:
                        cb()
                for cb in inject.values():
                    if cb:
                        cb()
                stage_swish(q)
            elif q == NQ:
                stage2(q - 1)
                stage3a(q - 2)
                stage3b(q - 3)
            elif q == NQ + 1:
                stage3a(q - 2)
                stage3b(q - 3)
            else:
                stage3b(q - 3)

    nc.compile()
    return nc


def _marshal(x, W1, b1, beta, W2, b2, cfg=DEFAULT_CFG):
    """Full inputs -> list of per-core input dicts."""
    fp8 = ml_dtypes.float8_e4m3
    # x: [1, B*X, L] -> [B, 7, 112, L] (g, c, p, l), cast once
    xg8 = np.asarray(x, dtype=np.float32).reshape(B, NCH, KC, L).astype(fp8)
    w1T = np.asarray(W1, dtype=np.float32).transpose(0, 2, 1)  # [B, X, Z]
    w1g = w1T.reshape(B, NCH, KC, Z)  # (g, c, p, z)
    w2s = np.asarray(W2, dtype=np.float32) * np.float32(1.0 / 1.1)  # [B, C, Z]
    b1f = np.asarray(b1, dtype=np.float32)  # [B, Z]
    b2f = np.asarray(b2, dtype=np.float32)  # [B, C]
    bf = np.asarray(beta, dtype=np.float32)  # [B]
    sph = np.log1p(np.exp(bf)) * np.float32(0.5)  # softplus(beta)/2

    pp = np.arange(P)
    maskb = (
        (pp[:, None] // 32 == pp[None, :] // 32) & (pp[:, None] % 32 < C)
    ).astype(ml_dtypes.bfloat16)

    in_maps = []
    for core in range(NCORE):
        s = slice(core * GPC, (core + 1) * GPC)
        # x -> (q, c, p, j, l)
        xq6 = (
            xg8[s].reshape(NQ, 4, NCH, KC, L).transpose(0, 2, 3, 1, 4)
        )  # [NQ, 7, 112, 4, 512]
        # half A: c0-2 (c,j,l) + c3 j0,1; half B: c3 j2,3 + c4-6
        a0 = xq6[:, 0:3].transpose(0, 2, 1, 3, 4).reshape(NQ, KC, 3 * 4 * L)
        a1 = xq6[:, 3, :, 0:2].reshape(NQ, KC, 2 * L)
        xam = np.concatenate([a0, a1], axis=2)  # [NQ, 112, 7168]
        b0 = xq6[:, 3, :, 2:4].reshape(NQ, KC, 2 * L)
        b1x = xq6[:, 4:7].transpose(0, 2, 1, 3, 4).reshape(NQ, KC, 3 * 4 * L)
        xbm = np.concatenate([b0, b1x], axis=2)  # [NQ, 112, 7168]
        # w1q[p, ((q*4+j)*7+c)*Z+z] = W1T[4q+j, 112c+p, z]
        wc = w1g[s].reshape(NQ, 4, NCH, KC, Z)
        w1qm = (
            wc.transpose(3, 0, 1, 2, 4).astype(fp8).reshape(KC, NQ * 4 * NCH * Z)
        )
        # w2q[32j+z, 32q+m] = W2[4q+j, m, z]/1.1 (m<C), else 0
        w2c = w2s[s].reshape(NQ, 4, C, Z)  # (q, j, m, z)
        w2qm = np.zeros((4, Z, NQ, 32), np.float16)
        w2qm[:, :, :, :C] = w2c.transpose(1, 3, 0, 2)
        w2qm = w2qm.reshape(P, NQ * 32)
        # per-partition scalars: [32j+z, q]
        b1qm = np.ascontiguousarray(
            b1f[s].reshape(NQ, 4, Z).transpose(1, 2, 0)
        ).reshape(P, NQ)
        sphqm = np.ascontiguousarray(
            np.broadcast_to(sph[s].reshape(NQ, 4).T[:, None, :], (4, Z, NQ))
        ).reshape(P, NQ)
        spb1hqm = sphqm * b1qm
        b2qm = np.full((4, 32, NQ), -30.0, np.float32)
        b2qm[:, :C, :] = b2f[s].reshape(NQ, 4, C).transpose(1, 2, 0)
        b2qm = b2qm.reshape(P, NQ)
        scalqm = np.concatenate([b1qm, sphqm, spb1hqm, b2qm], axis=1)
        in_maps.append(
            {
                "xa": xam,
                "xb": xbm,
                "w1q": w1qm,
                "w2q": w2qm,
                "scalq": scalqm,
                "maskb": maskb,
            }
        )
    return in_maps


def _run(in_maps, cfg=DEFAULT_CFG, trace=False, tmpdir=None):
    key = str(sorted(cfg.items()))
    if key not in _CACHE:
        _CACHE[key] = _build(cfg)
    return run_bass_kernel_spmd(
        _CACHE[key],
        in_maps,
        core_ids=list(range(NCORE)),
        trace=trace,
        tmpdir=tmpdir,
    )


_LAST = {}


def kernel(x, W1, b1, beta, W2, b2):
    cfg = dict(DEFAULT_CFG)
    ov = os.environ.get("KERNEL_CFG")
    if ov:
        for kv in ov.split(","):
            k, v = kv.split("=")
            cfg[k] = type(DEFAULT_CFG[k])(eval(v)) if not isinstance(
                DEFAULT_CFG[k], str
            ) else v
    in_maps = _marshal(x, W1, b1, beta, W2, b2, cfg)
    trace = bool(os.environ.get("KERNEL_TRACE"))
    r = _run(in_maps, cfg, trace=trace, tmpdir=os.environ.get("KERNEL_TRACE_DIR"))
    _LAST["results"] = r
    outs = [
        r.results[c]["out"].astype(np.float32).reshape(GPC, C * L)
        for c in range(NCORE)
    ]
    return np.concatenate(outs, axis=0)


# revision 6
# speedup vs baseline: 1.0194x; 1.0194x over previous
"""Grouped per-sample MLP (conv1d groups=B) + GroupSwish + softmax, on 8 NeuronCores.

Data-parallel over the group/batch axis B=256: 32 groups per core,
processed as 8 quads of 4 groups packed into the 128-partition dim.

Per group g: h = W1[g] @ x[g] + b1[g]; GroupSwish; o = W2[g] @ h + b2[g];
softmax over the flattened [C*L] logits.

The kernel is HBM-stream-bound (~290 GB/s/core under 8-core load, ~13.8MB
per core => ~48us stream floor); the design keeps the stream saturated
and the post-stream tail short:
  - x and W1 ship as fp8e4m3, swish output as fp16, out as bf16. End-to-
    end rel err ~9e-3 vs the 2e-2 gate.
  - The two DMA queues get x bytes in proportion to their measured
    service rates (SWDGE/gpsimd ~1.75x the HWDGE/sync rate when both are
    loaded), so both queues drain in lockstep and the last quad's data
    is not skewed onto one queue. Each quad's 28 512B (c,j) blocks are
    marshalled contiguous per partition and split 10 (sync) / 18
    (gpsimd); W1 splits 6/8 units the same way.
  - Each half is further split into granules (2 mid-stream, 4 on the
    last quad) so W1 matmuls consume x as it lands; after the final byte
    only the last granule's matmuls remain.
  - Emission order per iteration interleaves the previous quads' late
    stages between matmul granule groups, so in-order engine queues
    never park a ready instruction behind a stalled matmul:
    PE:  [mmG0(q)] [W2(q-1)] [mmG1(q)] [tot(q-2)] [mmG2(q)] [mmG3(q)]
    DVE: [recip(q-2)] [mul(q-3)]
    ACT: [exp(q-1)] [silu(q)]
  - GroupSwish is ONE activation: silu(sp*(h+b1)) with per-partition
    scale/bias, and the 1/(1.1*sp) factor folded into W2 host-side.
  - W2 is a single block-diagonal [128, 40] fp16 matmul whose output
    lands compactly at partitions 10j+m, so softmax runs on [40, L]
    with no padding and the store is ONE plain [40, 512] DMA per quad.
  - All of W1 stays resident in SBUF; softplus(beta) and all folding are
    host-side. Softmax denominators via one [40,40] block-mask matmul.
"""

import os
import ml_dtypes
import numpy as np
from contextlib import ExitStack

import concourse.mybir as mybir
import concourse.tile as tile
from concourse import bacc
from concourse.bass_utils import run_bass_kernel_spmd

B, X, Z, C, L = 256, 784, 32, 10, 512
NCORE = 8
GPC = B // NCORE  # 32 groups per core
NQ = GPC // 4  # 8 quads per core
KC = 112  # K-chunk size (7 * 112 = 784)
NCH = 7
P = 128
NB = NCH * 4  # 28 512B (c,j) blocks per quad per partition
F32 = mybir.dt.float32
F16 = mybir.dt.float16
F8 = mybir.dt.float8e4
BF16 = mybir.dt.bfloat16

DEFAULT_CFG = dict(
    x_bufs=4,
    s_bufs=4,
    h_bufs=3,
    o_bufs=2,
    sync_units=10,   # of 28 x blocks/quad on the sync (HWDGE) queue
    w1_sync_units=6,  # of 14 W1 units on the sync queue
    x_engines=("sync", "gpsimd"),
    out_engines=("gpsimd", "sync"),
    const_engine="gpsimd",
    mid_splits=2,   # granules per half for mid-stream quads
    last_splits=4,  # granules per half for the last quad
)

_CACHE: dict = {}


def _eng(nc, name):
    return getattr(nc, name)


def _build(cfg=DEFAULT_CFG):
    nc = bacc.Bacc("TRN2", target_bir_lowering=False, debug=False)

    SU = cfg["sync_units"]
    HBA = 512 * SU
    HBB = 512 * (NB - SU)

    xa = nc.dram_tensor("xa", [NQ, KC, HBA], F8, kind="ExternalInput").ap()
    xb = nc.dram_tensor("xb", [NQ, KC, HBB], F8, kind="ExternalInput").ap()
    w1q = nc.dram_tensor(
        "w1q", [KC, NQ * 4 * NCH * Z], F8, kind="ExternalInput"
    ).ap()
    # w2c[32j+z, 40q+10j+m] = W2[4q+j, m, z] / (1.1 * softplus(beta))
    w2q = nc.dram_tensor("w2q", [P, NQ * 40], F16, kind="ExternalInput").ap()
    # scal[:, 0:NQ]=softplus(beta), [NQ:2NQ]=sp*b1, [2NQ:3NQ]=b2 (compact, 40 rows)
    scalq = nc.dram_tensor("scalq", [P, 3 * NQ], F32, kind="ExternalInput").ap()
    # maskc[p, m] = 1 iff p//10 == m//10  (p, m < 40)
    maskb = nc.dram_tensor("maskb", [40, 40], BF16, kind="ExternalInput").ap()
    out = nc.dram_tensor("out", [GPC * C, L], BF16, kind="ExternalOutput").ap()

    # block (c, j) -> (half, byte offset); canonical block index 4c+j
    def cj_off(c, j):
        b = 4 * c + j
        return (0, 512 * b) if b < SU else (1, 512 * (b - SU))

    with tile.TileContext(nc) as tc, ExitStack() as ctx:
        consts = ctx.enter_context(tc.tile_pool(name="consts", bufs=1))
        xpool = ctx.enter_context(tc.tile_pool(name="x", bufs=cfg["x_bufs"]))
        spool = ctx.enter_context(tc.tile_pool(name="act", bufs=cfg["s_bufs"]))
        hps = ctx.enter_context(
            tc.tile_pool(name="hps", bufs=cfg["h_bufs"], space="PSUM")
        )
        ops = ctx.enter_context(
            tc.tile_pool(name="ops", bufs=cfg["o_bufs"], space="PSUM")
        )
        tps = ctx.enter_context(tc.tile_pool(name="tps", bufs=2, space="PSUM"))

        ce = _eng(nc, cfg["const_engine"])
        xes = [_eng(nc, e) for e in cfg["x_engines"]]
        oes = [_eng(nc, e) for e in cfg["out_engines"]]

        # W1 resident in SBUF, split across the queues in service-rate
        # proportion (free dim units of 512B).
        w1t = consts.tile([KC, NQ * 4 * NCH * Z], F8, name="w1t")
        wsplit = 512 * cfg["w1_sync_units"]
        xes[0].dma_start(w1t[:, :wsplit], w1q[:, :wsplit])
        xes[1].dma_start(w1t[:, wsplit:], w1q[:, wsplit:])
        w2t = consts.tile([P, NQ * 40], F16, name="w2t")
        ce.dma_start(w2t[:], w2q)
        scalt = consts.tile([P, 3 * NQ], F32, name="scalt")
        ce.dma_start(scalt[:], scalq)
        maskt = consts.tile([40, 40], BF16, name="maskt")
        ce.dma_start(maskt[:], maskb)
        spht = scalt[:, 0:NQ]
        spb1t = scalt[:, NQ : 2 * NQ]
        b2t = scalt[:, 2 * NQ : 3 * NQ]

        hqs, swishes, expos, esums, invcs = {}, {}, {}, {}, {}

        def w1s(q, j, c):
            k = (q * 4 + j) * NCH + c
            return w1t[:, k * Z : (k + 1) * Z]

        def granules(q):
            """Per-half granule boundaries (512-multiples) and the (c,j)
            matmuls bucketed by which granule holds their x."""
            n = cfg["last_splits"] if q == NQ - 1 else cfg["mid_splits"]
            bounds = []
            for hu in (SU, NB - SU):
                per = [hu // n + (1 if i < hu % n else 0) for i in range(n)]
                bb, acc = [], 0
                for u in per:
                    acc += u * 512
                    bb.append(acc)
                bounds.append(bb)
            groups = [[[] for _ in range(n)], [[] for _ in range(n)]]
            for c in range(NCH):
                for j in range(4):
                    h, off = cj_off(c, j)
                    gi = next(
                        i for i, bnd in enumerate(bounds[h]) if off + 512 <= bnd
                    )
                    groups[h][gi].append((c, j))
            return n, bounds, groups

        def stage1_dma(q):
            n, bounds, groups = granules(q)
            tiles = [[], []]
            src = [xa, xb]
            for h in range(2):
                lo = 0
                for gi in range(n):
                    hi = bounds[h][gi]
                    xt = xpool.tile(
                        [KC, hi - lo], F8, tag=f"x{h}_{gi}", name=f"x{q}_{h}_{gi}"
                    )
                    xes[h].dma_start(xt[:], src[h][q, :, lo:hi])
                    tiles[h].append((xt, lo))
                    lo = hi
            hq = hps.tile([P, L], F32, tag="h", name=f"h{q}")
            hqs[q] = hq
            return n, groups, tiles, hq

        def mm_block(q, h, gi, groups, tiles, hq, first, last):
            for c, j in groups[h][gi]:
                _, off = cj_off(c, j)
                xt, lo = tiles[h][gi]
                off -= lo
                nc.tensor.matmul(
                    hq[32 * j : 32 * j + 32, :],
                    w1s(q, j, c),
                    xt[:, off : off + 512],
                    start=((h, gi, c) == first[j]),
                    stop=((h, gi, c) == last[j]),
                    tile_position=(0, 32 * j),
                    skip_group_check=True,
                )

        def stage_swish(q):
            """GroupSwish as one op: silu(sp*(h+b1)); the 1/(1.1*sp)
            factor is folded into W2 host-side."""
            hq = hqs.pop(q)
            sw = spool.tile([P, L], F16, tag="sw", name=f"sw{q}")
            nc.scalar.activation(
                sw[:],
                hq[:],
                mybir.ActivationFunctionType.Silu,
                bias=spb1t[:, q : q + 1],
                scale=spht[:, q : q + 1],
            )
            swishes[q] = sw

        def stage2(q):
            """Block-diagonal W2 matmul (compact [40, L] output) + exp."""
            sw = swishes.pop(q)
            o = ops.tile([40, L], F32, tag="o", name=f"o{q}")
            nc.tensor.matmul(
                o[:],
                w2t[:, q * 40 : (q + 1) * 40],
                sw[:],
                start=True,
                stop=True,
            )
            expo = spool.tile([40, L], F32, tag="expo", name=f"e{q}")
            esum = spool.tile([40, 1], BF16, tag="esum", name=f"es{q}")
            with nc.allow_low_precision(reason="softmax denom, 2e-2 gate"):
                nc.scalar.activation(
                    expo[:],
                    o[:],
                    mybir.ActivationFunctionType.Exp,
                    bias=b2t[0:40, q : q + 1],
                    scale=1.0,
                    accum_out=esum[:],
                )
            expos[q] = expo
            esums[q] = esum

        def stage3a(q):
            """Per-group exp totals + reciprocal."""
            esum = esums.pop(q)
            tot = tps.tile([40, 1], F32, tag="tot", name=f"tot{q}")
            nc.tensor.matmul(tot[:], maskt[:], esum[:], start=True, stop=True)
            invc = spool.tile([40, 1], F32, tag="invc", name=f"ic{q}")
            nc.vector.reciprocal(invc[:], tot[:])
            invcs[q] = invc

        def stage3b(q):
            """Normalize + store: one plain [40, 512] DMA."""
            invc = invcs.pop(q)
            expo = expos.pop(q)
            res = spool.tile([40, L], BF16, tag="res", name=f"r{q}")
            nc.vector.tensor_scalar_mul(res[:], expo[:], invc[:])
            oes[q % 2].dma_start(out[40 * q : 40 * q + 40], res[:])

        for q in range(NQ + 3):
            if q < NQ:
                n, groups, tiles, hq = stage1_dma(q)
                order = [(h, gi) for gi in range(n) for h in range(2)]
                first, last = {}, {}
                for h, gi in order:
                    for c, j in groups[h][gi]:
                        key = (h, gi, c)
                        if j not in first:
                            first[j] = key
                        last[j] = key
                inject = {
                    1: (lambda: stage2(q - 1)) if q >= 1 else None,
                    2: (lambda: stage3a(q - 2)) if q >= 2 else None,
                    3: (lambda: stage3b(q - 3)) if q >= 3 else None,
                }
                for bi, (h, gi) in enumerate(order):
                    mm_block(q, h, gi, groups, tiles, hq, first, last)
                    cb = inject.pop(bi + 1, None)
                    if cb:
                        cb()
                for cb in inject.values():
                    if cb:
                        cb()
                stage_swish(q)
            elif q == NQ:
                stage2(q - 1)
                stage3a(q - 2)
                stage3b(q - 3)
            elif q == NQ + 1:
                stage3a(q - 2)
                stage3b(q - 3)
            else:
                stage3b(q - 3)

    nc.compile()
    return nc


def _marshal(x, W1, b1, beta, W2, b2, cfg=DEFAULT_CFG):
    """Full inputs -> list of per-core input dicts."""
    fp8 = ml_dtypes.float8_e4m3
    SU = cfg["sync_units"]
    # x: [1, B*X, L] -> [B, 7, 112, L] (g, c, p, l), cast once
    xg8 = np.asarray(x, dtype=np.float32).reshape(B, NCH, KC, L).astype(fp8)
    w1T = np.asarray(W1, dtype=np.float32).transpose(0, 2, 1)  # [B, X, Z]
    w1g = w1T.reshape(B, NCH, KC, Z)  # (g, c, p, z)
    b1f = np.asarray(b1, dtype=np.float32)  # [B, Z]
    b2f = np.asarray(b2, dtype=np.float32)  # [B, C]
    bf = np.asarray(beta, dtype=np.float32)  # [B]
    sph = np.log1p(np.exp(bf)).astype(np.float32)  # softplus(beta), [B]
    # W2 / (1.1 * sp) : [B, C, Z]
    w2s = np.asarray(W2, dtype=np.float32) / (1.1 * sph)[:, None, None]

    pp = np.arange(40)
    maskb = (pp[:, None] // C == pp[None, :] // C).astype(ml_dtypes.bfloat16)

    in_maps = []
    for core in range(NCORE):
        s = slice(core * GPC, (core + 1) * GPC)
        # x -> (q, p, c, j, l) flattened; split blocks at SU
        xfull = (
            xg8[s]
            .reshape(NQ, 4, NCH, KC, L)
            .transpose(0, 3, 2, 1, 4)
            .reshape(NQ, KC, NB * 512)
        )
        xam = np.ascontiguousarray(xfull[:, :, : 512 * SU])
        xbm = np.ascontiguousarray(xfull[:, :, 512 * SU :])
        # w1q[p, ((q*4+j)*7+c)*Z+z] = W1T[4q+j, 112c+p, z]
        wc = w1g[s].reshape(NQ, 4, NCH, KC, Z)
        w1qm = (
            wc.transpose(3, 0, 1, 2, 4).astype(fp8).reshape(KC, NQ * 4 * NCH * Z)
        )
        # w2c[32j+z, 40q+10j+m] = w2s[4q+j, m, z]
        w2c = w2s[s].reshape(NQ, 4, C, Z)  # (q, j, m, z)
        w2qm = np.zeros((4, Z, NQ, 4, C), np.float16)
        for j in range(4):
            w2qm[j, :, :, j, :] = w2c[:, j].transpose(2, 0, 1)  # (z, q, m)
        w2qm = w2qm.reshape(P, NQ * 40)
        # per-partition scalars
        sphq = np.ascontiguousarray(
            np.broadcast_to(
                sph[s].reshape(NQ, 4).T[:, None, :], (4, Z, NQ)
            )
        ).reshape(P, NQ)
        b1q = np.ascontiguousarray(
            b1f[s].reshape(NQ, 4, Z).transpose(1, 2, 0)
        ).reshape(P, NQ)
        spb1q = sphq * b1q
        b2q = np.zeros((P, NQ), np.float32)
        b2q[0:40] = (
            b2f[s].reshape(NQ, 4, C).transpose(1, 2, 0).reshape(40, NQ)
        )
        scalqm = np.concatenate([sphq, spb1q, b2q], axis=1)
        in_maps.append(
            {
                "xa": xam,
                "xb": xbm,
                "w1q": w1qm,
                "w2q": w2qm,
                "scalq": scalqm,
                "maskb": maskb,
            }
        )
    return in_maps


def _run(in_maps, cfg=DEFAULT_CFG, trace=False, tmpdir=None):
    key = str(sorted(cfg.items()))
    if key not in _CACHE:
        _CACHE[key] = _build(cfg)
    return run_bass_kernel_spmd(
        _CACHE[key],
        in_maps,
        core_ids=list(range(NCORE)),
        trace=trace,
        tmpdir=tmpdir,
    )


_LAST = {}


def kernel(x, W1, b1, beta, W2, b2):
    cfg = dict(DEFAULT_CFG)
    ov = os.environ.get("KERNEL_CFG")
    if ov:
        for kv in ov.split(","):
            k, v = kv.split("=")
            cfg[k] = type(DEFAULT_CFG[k])(eval(v)) if not isinstance(
                DEFAULT_CFG[k], str
            ) else v
    in_maps = _marshal(x, W1, b1, beta, W2, b2, cfg)
    trace = bool(os.environ.get("KERNEL_TRACE"))
    r = _run(in_maps, cfg, trace=trace, tmpdir=os.environ.get("KERNEL_TRACE_DIR"))
    _LAST["results"] = r
    outs = [
        r.results[c]["out"].astype(np.float32).reshape(GPC, C * L)
        for c in range(NCORE)
    ]
    return np.concatenate(outs, axis=0)
